# revision 47
# baseline (speedup 1.0000x reference)
"""BiLinearAttention TRN2 Bass kernel.

Math (per batch element n, data-parallel over 8 NeuronCores):
    q_proj = query @ W.T + b          # [L, D]
    score  = q_proj @ key.T           # [L, S]
    P      = softmax(score, axis=-1)
    out    = P @ value                # [L, D]

Shapes: query/key/value [2048, 1024] f32 per core, W [1024, 1024], b [1024].

Design notes:
  - Single-pass fp16 matmuls everywhere (1 cycle/row on the PE vs 4 for
    fp32). Rounding all operands to fp16 injects ~0.017 std of logit noise
    (numpy-sim on the real inputs), which softmax turns into 2.5e-3 output
    rel err -- an 8x margin under the 2e-2 gate. The earlier 3-pass fp16
    hi/lo split scheme (2.1e-4) spends 2.2x the PE cycles buying accuracy
    that isn't needed. bf16 (8-bit mantissa, ~8x the logit noise) is NOT
    safe here: score std ~45 with top-2 gaps ~11 makes softmax a
    near-argmax and bf16 visibly corrupts the output.
  - PE floor: proj 131072 + score 262144 + PV 262144 = 655360 cycles
    (273 us at 2.4 GHz).
  - Every DMA instruction costs ~3-4 us end-to-end on its queue (trigger
    + DGE handoff + device + semaphore) regardless of size, so prep is
    BATCHED: all input loads are gpsimd cast-DMAs (f32 HBM -> f16 SBUF
    in flight, 4 row-tiles per instruction) and every X-bar transpose
    moves a whole [128, 4x1024] group in one descriptor set. 12 loads +
    10 transposes total, vs 88 per-row-tile instructions.
  - No PE transposes: operands reach contraction-major layout via the
    2-byte X-bar DMA transpose. ALL X-bars stay on the SP HWDGE queue
    (concurrent X-bar streams from two HWDGE queues corrupt data --
    HW-verified earlier); the batched group layout [128, t, kc, 128]
    feeds matmuls directly as 3D access patterns.
  - Schedule: proj lb0 is the only compute gated on prep (starts ~13us);
    proj lb1-3 and the Q-block prep for lb2/3 interleave into the
    attention pipeline (phase C) so the PE never waits on them. K is the
    gate for the first score tile (~36us); V arrives before the first PV.
  - Softmax over s in [l, s] layout: free-dim reduce_max on DVE, exp on
    ACT reading score PSUM directly, with accum_out producing the
    denominator. P is emitted as fp16 scaled by 2^10 (folded into the
    exp bias; the normalizer absorbs it) to keep the tail of the
    near-one-hot distribution out of fp16 denormals.
  - P tiles X-bar-transposed, P.T @ value in fp16, then
    out = psum * (1/sum) via per-partition tensor_scalar on DVE.
"""

import numpy as np
from contextlib import ExitStack

import concourse.bass as bass
import concourse.tile as tile
from concourse import mybir, bacc, bass_utils
from concourse.masks import make_identity

F32 = mybir.dt.float32
F16 = mybir.dt.float16
AF = mybir.ActivationFunctionType
AX = mybir.AxisListType

N, L, S, D = 8, 2048, 2048, 1024
N_CORES = 8
LT = L // 128       # 16 l tiles
ST = S // 128       # 16 s tiles
KC = D // 128       # 8 contraction chunks (both q and k dims)
SB = S // 512       # 4 score blocks per l tile
LB = L // 512       # 4 l blocks in projection
DB = D // 512       # 2 d blocks in PV

PSCALE = float(np.log(1024.0))


def _emit(ctx: ExitStack, tc: tile.TileContext,
          query, key, value, W, b, out, loop_T=0):
    nc = tc.nc
    _emit.uid = getattr(_emit, "uid", 0)

    base = ctx.enter_context(tc.tile_pool(name="base", bufs=1))
    b_sb = base.tile([128, KC], F32)
    nc.gpsimd.dma_start(b_sb, b.rearrange("(t p) -> p t", p=128))
    ident = base.tile([128, 128], F16)
    make_identity(nc, ident)

    # persistent transposed fp16 operands. K and V load with rows folded
    # contiguously per partition ("(p t) d": partition p holds rows
    # 8p..8p+7 -- ONE descriptor per partition, 128 per DMA, so the
    # 1024-slot SWDGE ring never stalls on them). This scrambles the
    # s-order: softmax is order-invariant over s and PV re-pairs s via
    # v16 slicing (pt chunk sc <-> v16[sc//8][:, sc%8, :]). W and Q keep
    # the row-per-partition "(t p)" layout (512 desc) because their row
    # indices become the k / l output orders, which must stay linear.
    # WT[h]:  [q', kt_in_half, qc, k']  = W[(4h+t)*128+k', qc*128+q']
    # qpT[lb]: [k', kt, l512]           = q_proj[lb*512+l, kt*128+k']
    # kT[h]:  [k', t, kc, c]            = key[h*1024+8c+t, kc*128+k']
    # v16[h]: [s', u, d]                = value[h*1024+8s'+u, d]
    p_w = ctx.enter_context(tc.tile_pool(name="wt", bufs=1))
    WT = [p_w.tile([128, 4, KC, 128], F16, name=f"WT{h}") for h in range(2)]
    p_qp = ctx.enter_context(tc.tile_pool(name="qp", bufs=1))
    qpT = [p_qp.tile([128, KC, 512], F16, name=f"qpT{i}") for i in range(LB)]
    p_kv = ctx.enter_context(tc.tile_pool(name="kv", bufs=1))
    kT = [p_kv.tile([128, 8, KC, 128], F16, name=f"kT{i}") for i in range(2)]
    v16 = [p_kv.tile([128, 8, D], F16, name=f"v16_{i}") for i in range(2)]

    # fp16 natural-layout staging (written by gpsimd cast-loads, consumed
    # by one batched X-bar each). DMA cost structure (cost-model + HW):
    # consecutive same-kind DMAs pipeline gap-free, but every load<->X-bar
    # transition costs ~2.5us of dead DMA time. So prep is ONE run of
    # loads, then ONE run of X-bars, then V.
    p_qn = ctx.enter_context(tc.tile_pool(name="qn", bufs=2))
    p_qtb = ctx.enter_context(tc.tile_pool(name="qtb", bufs=2))
    ps_mm = ctx.enter_context(tc.tile_pool(name="ps_mm", bufs=2, space="PSUM"))

    def cast_load(pool, tag, src_rows, nt=4, fold=False):
        """nt*128 consecutive rows -> [128,nt,D] f16 gpsimd cast-DMA.
        fold=True: rows contiguous per partition (128 descriptors);
        fold=False: row-per-partition tiles (nt/4 * 512 descriptors)."""
        t = pool.tile([128, nt, D], F16, tag=tag, name=f"{tag}{_emit.uid}")
        _emit.uid += 1
        if fold:
            nc.gpsimd.dma_start(t, src_rows.rearrange("(p t) d -> p t d", t=nt))
        else:
            nc.gpsimd.dma_start(t, src_rows.rearrange("(t p) d -> p t d", p=128))
        return t

    qT_tiles = {}

    def pe_transpose(dst4, src, nt, ps_pool, ps_tag):
        """Transpose [128, nt, D] f16 natural tile into [128, nt, KC, 128]
        contraction-major via PE transpose matmuls (1 cycle/row; the PE is
        the one engine with prep slack). 4 chunk-transposes pack one
        [128,4,128] f16 PSUM tile; one copy drains it to SBUF."""
        for t in range(nt):
            for j in range(2):
                ps = ps_pool.tile([128, 4, 128], F16, tag=ps_tag)
                for i in range(4):
                    qc = 4 * j + i
                    nc.tensor.transpose(ps[:, i, :],
                                        src[:, t, qc * 128:(qc + 1) * 128],
                                        ident)
                nc.any.tensor_copy(dst4[:, t, 4 * j:4 * j + 4, :], ps)

    def emit_proj(lb):
        """q_projT[k, l_blk] = sum_q W.T[q, k] @ queryT[q, l_blk]; +b -> f16"""
        qT = qT_tiles[lb]
        for kt in range(KC):
            mm = ps_mm.tile([128, 512], F32, tag="mm")
            for qc in range(KC):
                nc.tensor.matmul(mm, WT[kt // 4][:, kt % 4, qc, :],
                                 qT[:, :, qc, :],
                                 start=(qc == 0), stop=(qc == KC - 1))
            nc.scalar.activation(qpT[lb][:, kt, :], mm, AF.Identity,
                                 bias=b_sb[:, kt:kt + 1], scale=1.0)

    with tc.tile_pool(name="kn", bufs=2) as p_kn, \
         tc.tile_pool(name="wn", bufs=2) as p_wn, \
         tc.tile_pool(name="ps_tr", bufs=4, space="PSUM") as ps_tr:
        # loads (gpsimd, ~1us each even fully serialized): W, Q0, K, Q1, V
        w16 = [cast_load(p_wn, "w16", W[h * 512:(h + 1) * 512, :])
               for h in range(2)]
        q16 = {0: cast_load(p_qn, "q16", query[0:512, :])}
        k16 = [cast_load(p_kn, "k16", key[h * 1024:(h + 1) * 1024, :],
                         nt=8, fold=True)
               for h in range(2)]
        q16[1] = cast_load(p_qn, "q16", query[512:1024, :])
        for h in range(2):
            nc.gpsimd.dma_start(
                v16[h],
                value[h * 1024:(h + 1) * 1024, :].rearrange("(p t) d -> p t d", t=8))

        # PE transposes + proj: W, Q0 -> proj lb0 -> K -> (score starts)
        def tr_q(lb, ps_pool, ps_tag):
            t = p_qtb.tile([128, 4, KC, 128], F16, tag="qT",
                           name=f"qT{_emit.uid}")
            _emit.uid += 1
            pe_transpose(t, q16[lb], 4, ps_pool, ps_tag)
            qT_tiles[lb] = t

        for h in range(2):
            pe_transpose(WT[h], w16[h], 4, ps_tr, "tr")
        tr_q(0, ps_tr, "tr")
        emit_proj(0)
        for h in range(2):
            pe_transpose(kT[h], k16[h], 8, ps_tr, "tr")

    # ------- phase C: attention over l tiles -------
    ps_score = ctx.enter_context(tc.tile_pool(name="ps_s", bufs=4, space="PSUM"))
    ps_out = ctx.enter_context(tc.tile_pool(name="ps_o", bufs=2, space="PSUM"))
    p_p = ctx.enter_context(tc.tile_pool(name="p_p", bufs=2))
    p_pt = ctx.enter_context(tc.tile_pool(name="p_pt", bufs=2))
    p_stat = ctx.enter_context(tc.tile_pool(name="p_stat", bufs=3))
    p_out = ctx.enter_context(tc.tile_pool(name="p_out", bufs=2))

    def emit_score_softmax(lt):
        """Score matmuls + softmax for l tile lt; returns (PT, 1/sum).
        P column j=t'*128+c holds s=(sb//2)*1024+8c+(sb%2)*4+t' (the
        K fold scrambles s; softmax is order-invariant over s)."""
        score_ps = []
        mx4 = p_stat.tile([128, SB], F32, tag="mx4")
        lb, li = divmod(lt, 4)
        lsl = slice(li * 128, (li + 1) * 128)
        # kc-outer: one stationary (qpT chunk) serves all 4 s-blocks --
        # 8 LDWEIGHTS per l-tile instead of 32
        for sb in range(SB):
            sc_mm = ps_score.tile([128, 512], F32, tag="sc")
            score_ps.append(sc_mm)
        for kc in range(KC):
            for sb in range(SB):
                tsl = slice((sb % 2) * 4, (sb % 2) * 4 + 4)
                nc.tensor.matmul(score_ps[sb], qpT[lb][:, kc, lsl],
                                 kT[sb // 2][:, tsl, kc, :],
                                 start=(kc == 0), stop=(kc == KC - 1))
        for sb in range(SB):
            nc.vector.reduce_max(mx4[:, sb:sb + 1], score_ps[sb], axis=AX.X)

        nm = p_stat.tile([128, 1], F32, tag="nm")
        # nm = -(max) + ln(2^10): P scaled by 1024 (normalizer absorbs it)
        nc.vector.reduce_max(nm, mx4, axis=AX.X, negate=True)
        nc.vector.tensor_scalar_add(nm, nm, PSCALE)
        p_sb = p_p.tile([128, S], F16, tag="p")
        ssum4 = p_stat.tile([128, SB], F32, tag="ssum4")
        for sb in range(SB):
            nc.scalar.activation(p_sb[:, sb * 512:(sb + 1) * 512], score_ps[sb],
                                 AF.Exp, bias=nm, scale=1.0,
                                 accum_out=ssum4[:, sb:sb + 1])
        ssum = p_stat.tile([128, 1], F32, tag="ssum")
        nc.vector.reduce_sum(ssum, ssum4, axis=AX.X)
        rinv = p_stat.tile([128, 1], F32, tag="rinv")
        nc.vector.reciprocal(rinv, ssum)
        # PT[s', sc, l'] = P[l', sc*128+s'] -- one batched xbar transpose
        pt = p_pt.tile([128, ST, 128], F16, tag="pt")
        nc.sync.dma_start(pt, p_sb, transpose=True)
        return pt, rinv

    def emit_pv(lt, pt, rinv):
        """P.T-weighted V accumulation, scale, store. PT chunk sc pairs
        with value rows 8p+sc%8 of half sc//8 (= v16 layout exactly)."""
        out_ps = [ps_out.tile([128, 512], F32, tag="o", name=f"ops{lt}_{i}")
                  for i in range(DB)]
        for sc in range(ST):
            for dc in range(DB):
                nc.tensor.matmul(out_ps[dc], pt[:, sc, :],
                                 v16[sc // 8][:, sc % 8, dc * 512:(dc + 1) * 512],
                                 start=(sc == 0), stop=(sc == ST - 1))
        o_sb = p_out.tile([128, D], F32, tag="osb")
        for dc in range(DB):
            nc.vector.tensor_scalar_mul(o_sb[:, dc * 512:(dc + 1) * 512],
                                        out_ps[dc], rinv)
        nc.gpsimd.dma_start(out[lt * 128:(lt + 1) * 128, :], o_sb)

    def phase4():
        # proj lb1-3 + their qT PE-transposes interleave into the attention
        # pipeline a few tiles ahead of the first score tile that reads
        # them; lb2/lb3 Q loads reuse staging slots freed by earlier
        # transposes (WAR dep needs the reader emitted first).
        pending = None
        for lt in range(LT):
            cur = emit_score_softmax(lt)
            if pending is not None:
                emit_pv(lt - 1, *pending)
            pending = cur
            if lt == 0:
                tr_q(1, ps_mm, "mm")
                emit_proj(1)
            if lt in (0, 4):
                lb = lt // 4 + 2
                q16[lb] = cast_load(p_qn, "q16",
                                    query[lb * 512:(lb + 1) * 512, :])
            if lt in (2, 6):
                tr_q(lt // 4 + 2, ps_mm, "mm")
            if lt in (5, 9):
                emit_proj(lt // 4 + 1)
        emit_pv(LT - 1, *pending)

    if loop_T:
        with tc.For_i(0, loop_T, 1):
            phase4()
    else:
        phase4()


_CACHE = {}


def _build(reps=1, loop_T=0, loop_all=0):
    key_ = (reps, loop_T, loop_all)
    if key_ in _CACHE:
        return _CACHE[key_]
    nc = bacc.Bacc("TRN2", target_bir_lowering=False, debug=False,
                   num_devices=N_CORES)
    query = nc.dram_tensor("query", [L, D], F32, kind="ExternalInput").ap()
    key = nc.dram_tensor("key", [S, D], F32, kind="ExternalInput").ap()
    value = nc.dram_tensor("value", [S, D], F32, kind="ExternalInput").ap()
    W = nc.dram_tensor("W", [D, D], F32, kind="ExternalInput").ap()
    b = nc.dram_tensor("b", [D], F32, kind="ExternalInput").ap()
    out = nc.dram_tensor("out", [L, D], F32, kind="ExternalOutput").ap()
    tag = None
    loop_T = loop_T or loop_all
    if reps > 1 or loop_T:
        # distinct I/O signature per variant so the neuron compile cache
        # (keyed on HLO structure, not backend_config) can't collide
        tag = nc.dram_tensor("tag", [8, reps * 100 + max(loop_T, 1)], F32,
                             kind="ExternalOutput").ap()
    with tile.TileContext(nc) as tc:
        if loop_all:
            with tc.For_i(0, loop_all, 1):
                with ExitStack() as ctx:
                    _emit(ctx, tc, query, key, value, W, b, out)
        else:
            for _ in range(reps):
                with ExitStack() as ctx:
                    _emit(ctx, tc, query, key, value, W, b, out, loop_T=loop_T)
        if tag is not None:
            with tc.tile_pool(name="tagp", bufs=1) as tp:
                t = tp.tile([8, reps * 100 + max(loop_T, 1)], F32)
                nc.vector.memset(t, 1.0)
                nc.sync.dma_start(tag, t)
    nc.compile()
    _CACHE[key_] = nc
    return nc


def kernel(key, query, value, W, b):
    key = np.ascontiguousarray(np.asarray(key), dtype=np.float32)
    query = np.ascontiguousarray(np.asarray(query), dtype=np.float32)
    value = np.ascontiguousarray(np.asarray(value), dtype=np.float32)
    W = np.ascontiguousarray(np.asarray(W), dtype=np.float32)
    b = np.ascontiguousarray(np.asarray(b), dtype=np.float32)
    nc = _build()
    in_maps = [
        {"query": query[i], "key": key[i], "value": value[i], "W": W, "b": b}
        for i in range(N_CORES)
    ]
    res = bass_utils.run_bass_kernel_spmd(nc, in_maps, core_ids=list(range(N_CORES)))
    return np.stack([res.results[i]["out"] for i in range(N_CORES)], axis=0)


# revision 50
# speedup vs baseline: 1.1205x; 1.1205x over previous
"""BiLinearAttention TRN2 Bass kernel.

Math (per batch element n, data-parallel over 8 NeuronCores):
    q_proj = query @ W.T + b          # [L, D]
    score  = q_proj @ key.T           # [L, S]
    P      = softmax(score, axis=-1)
    out    = P @ value                # [L, D]

Shapes: query/key/value [2048, 1024] f32 per core, W [1024, 1024], b [1024].

Design notes:
  - Single-pass fp16 matmuls everywhere (1 cycle/row on the PE vs 4 for
    fp32). Rounding all operands to fp16 injects ~0.017 std of logit noise
    (numpy-sim on the real inputs), which softmax turns into 2.5e-3 output
    rel err -- an 8x margin under the 2e-2 gate. The earlier 3-pass fp16
    hi/lo split scheme (2.1e-4) spends 2.2x the PE cycles buying accuracy
    that isn't needed. bf16 (8-bit mantissa, ~8x the logit noise) is NOT
    safe here: score std ~45 with top-2 gaps ~11 makes softmax a
    near-argmax and bf16 visibly corrupts the output.
  - PE floor: proj 131072 + score 262144 + PV 262144 = 655360 cycles
    (273 us at 2.4 GHz).
  - The Tile framework serializes ALL DMAs on one chain (each DMA's
    issue waits the previous DMA's completion semaphore -- HW-trace
    verified), and an X-bar DMA transpose costs 4.8-9.6us. So W/Q/K
    reach contraction-major layout via PE transpose matmuls instead
    (1 cycle/row fp16, ~17us on the PE which is otherwise idle during
    prep), packed 4 chunks per fp16 PSUM tile and drained by DVE/ACT.
    Only the per-tile P transpose stays on the SP X-bar (the PE has no
    slack in phase C; concurrent X-bar streams from two HWDGE queues
    corrupt data, so all X-bars stay on SP).
  - All input loads are gpsimd cast-DMAs (f32 HBM -> f16 SBUF in
    flight). K and V fold 8 rows per partition ("(p t) d", 128
    descriptors -- the 1024-slot SWDGE ring never stalls); the s-order
    scramble this causes is absorbed by softmax order-invariance and
    re-paired in PV via v16[sc//8][:, sc%8]. W/Q keep row-per-partition
    order (their row index becomes the linear k/l output order).
  - Schedule: PE does W/Q0/K transposes then proj lb0 (~10us in); score
    tiles start as soon as kT lands. proj lb1-3 + qT transposes for
    lb1-3 interleave into the attention pipeline. Out stores pair two
    l-tiles per DMA to halve phase-C links in the serial DMA chain.
  - Score runs kc-outer so one stationary LDWEIGHTS serves all 4
    s-blocks (8 instead of 32 per l-tile).
  - Softmax over s in [l, s] layout: free-dim reduce_max on DVE, exp on
    ACT reading score PSUM directly, with accum_out producing the
    denominator. P is emitted as fp16 scaled by 2^10 (folded into the
    exp bias; the normalizer absorbs it) to keep the tail of the
    near-one-hot distribution out of fp16 denormals.
  - P tiles X-bar-transposed, P.T @ value in fp16, then
    out = psum * (1/sum) via per-partition tensor_scalar on DVE.
"""

import numpy as np
from contextlib import ExitStack

import concourse.bass as bass
import concourse.tile as tile
from concourse import mybir, bacc, bass_utils
from concourse.masks import make_identity

F32 = mybir.dt.float32
F16 = mybir.dt.float16
AF = mybir.ActivationFunctionType
AX = mybir.AxisListType

N, L, S, D = 8, 2048, 2048, 1024
N_CORES = 8
LT = L // 128       # 16 l tiles
ST = S // 128       # 16 s tiles
KC = D // 128       # 8 contraction chunks (both q and k dims)
SB = S // 512       # 4 score blocks per l tile
LB = L // 512       # 4 l blocks in projection
DB = D // 512       # 2 d blocks in PV

PSCALE = float(np.log(1024.0))


def _emit(ctx: ExitStack, tc: tile.TileContext,
          query, key, value, W, b, out, loop_T=0):
    nc = tc.nc
    _emit.uid = getattr(_emit, "uid", 0)

    base = ctx.enter_context(tc.tile_pool(name="base", bufs=1))
    b_sb = base.tile([128, KC], F32)
    nc.gpsimd.dma_start(b_sb, b.rearrange("(t p) -> p t", p=128))
    ident = base.tile([128, 128], F16)
    make_identity(nc, ident)

    # persistent transposed fp16 operands. K and V load with rows folded
    # contiguously per partition ("(p t) d": partition p holds rows
    # 8p..8p+7 -- ONE descriptor per partition, 128 per DMA, so the
    # 1024-slot SWDGE ring never stalls on them). This scrambles the
    # s-order: softmax is order-invariant over s and PV re-pairs s via
    # v16 slicing (pt chunk sc <-> v16[sc//8][:, sc%8, :]). W and Q keep
    # the row-per-partition "(t p)" layout (512 desc) because their row
    # indices become the k / l output orders, which must stay linear.
    # WT[h]:  [q', kt_in_half, qc, k']  = W[(4h+t)*128+k', qc*128+q']
    # qpT[lb]: [k', kt, l512]           = q_proj[lb*512+l, kt*128+k']
    # kT[h]:  [k', t, kc, c]            = key[h*1024+8c+t, kc*128+k']
    # v16[h]: [s', u, d]                = value[h*1024+8s'+u, d]
    p_w = ctx.enter_context(tc.tile_pool(name="wt", bufs=1))
    WT = [p_w.tile([128, 4, KC, 128], F16, name=f"WT{h}") for h in range(2)]
    p_qp = ctx.enter_context(tc.tile_pool(name="qp", bufs=1))
    qpT = [p_qp.tile([128, KC, 512], F16, name=f"qpT{i}") for i in range(LB)]
    p_kv = ctx.enter_context(tc.tile_pool(name="kv", bufs=1))
    kT = [p_kv.tile([128, 8, KC, 128], F16, name=f"kT{i}") for i in range(2)]
    v16 = [p_kv.tile([128, 8, D], F16, name=f"v16_{i}") for i in range(2)]

    # fp16 natural-layout staging (written by gpsimd cast-loads, consumed
    # by one batched X-bar each). DMA cost structure (cost-model + HW):
    # consecutive same-kind DMAs pipeline gap-free, but every load<->X-bar
    # transition costs ~2.5us of dead DMA time. So prep is ONE run of
    # loads, then ONE run of X-bars, then V.
    p_qn = ctx.enter_context(tc.tile_pool(name="qn", bufs=2))
    p_qtb = ctx.enter_context(tc.tile_pool(name="qtb", bufs=2))
    ps_mm = ctx.enter_context(tc.tile_pool(name="ps_mm", bufs=2, space="PSUM"))

    def cast_load(pool, tag, src_rows, nt=4, fold=False):
        """nt*128 consecutive rows -> [128,nt,D] f16 gpsimd cast-DMA.
        fold=True: rows contiguous per partition (128 descriptors);
        fold=False: row-per-partition tiles (nt/4 * 512 descriptors)."""
        t = pool.tile([128, nt, D], F16, tag=tag, name=f"{tag}{_emit.uid}")
        _emit.uid += 1
        if fold:
            nc.gpsimd.dma_start(t, src_rows.rearrange("(p t) d -> p t d", t=nt))
        else:
            nc.gpsimd.dma_start(t, src_rows.rearrange("(t p) d -> p t d", p=128))
        return t

    qT_tiles = {}

    def pe_transpose(dst4, src, nt, ps_pool, ps_tag):
        """Transpose [128, nt, D] f16 natural tile into [128, nt, KC, 128]
        contraction-major via PE transpose matmuls (1 cycle/row; the PE is
        the one engine with prep slack). 4 chunk-transposes pack one
        [128,4,128] f16 PSUM tile; one copy drains it to SBUF."""
        for t in range(nt):
            for j in range(2):
                ps = ps_pool.tile([128, 4, 128], F16, tag=ps_tag)
                for i in range(4):
                    qc = 4 * j + i
                    nc.tensor.transpose(ps[:, i, :],
                                        src[:, t, qc * 128:(qc + 1) * 128],
                                        ident)
                nc.any.tensor_copy(dst4[:, t, 4 * j:4 * j + 4, :], ps)

    def emit_proj(lb):
        """q_projT[k, l_blk] = sum_q W.T[q, k] @ queryT[q, l_blk]; +b -> f16"""
        qT = qT_tiles[lb]
        for kt in range(KC):
            mm = ps_mm.tile([128, 512], F32, tag="mm")
            for qc in range(KC):
                nc.tensor.matmul(mm, WT[kt // 4][:, kt % 4, qc, :],
                                 qT[:, :, qc, :],
                                 start=(qc == 0), stop=(qc == KC - 1))
            nc.scalar.activation(qpT[lb][:, kt, :], mm, AF.Identity,
                                 bias=b_sb[:, kt:kt + 1], scale=1.0)

    with tc.tile_pool(name="kn", bufs=2) as p_kn, \
         tc.tile_pool(name="wn", bufs=2) as p_wn, \
         tc.tile_pool(name="ps_tr", bufs=4, space="PSUM") as ps_tr:
        # loads (gpsimd, ~1us each even fully serialized): W, Q0, K, Q1, V
        w16 = [cast_load(p_wn, "w16", W[h * 512:(h + 1) * 512, :])
               for h in range(2)]
        q16 = {0: cast_load(p_qn, "q16", query[0:512, :])}
        k16 = [cast_load(p_kn, "k16", key[h * 1024:(h + 1) * 1024, :],
                         nt=8, fold=True)
               for h in range(2)]
        q16[1] = cast_load(p_qn, "q16", query[512:1024, :])
        for h in range(2):
            nc.gpsimd.dma_start(
                v16[h],
                value[h * 1024:(h + 1) * 1024, :].rearrange("(p t) d -> p t d", t=8))

        # PE transposes + proj: W, Q0 -> proj lb0 -> K -> (score starts)
        def tr_q(lb, ps_pool, ps_tag):
            t = p_qtb.tile([128, 4, KC, 128], F16, tag="qT",
                           name=f"qT{_emit.uid}")
            _emit.uid += 1
            pe_transpose(t, q16[lb], 4, ps_pool, ps_tag)
            qT_tiles[lb] = t

        for h in range(2):
            pe_transpose(WT[h], w16[h], 4, ps_tr, "tr")
        tr_q(0, ps_tr, "tr")
        for h in range(2):
            pe_transpose(kT[h], k16[h], 8, ps_tr, "tr")
        emit_proj(0)

    # ------- phase C: attention over l tiles -------
    ps_score = ctx.enter_context(tc.tile_pool(name="ps_s", bufs=4, space="PSUM"))
    ps_out = ctx.enter_context(tc.tile_pool(name="ps_o", bufs=2, space="PSUM"))
    p_p = ctx.enter_context(tc.tile_pool(name="p_p", bufs=2))
    p_pt = ctx.enter_context(tc.tile_pool(name="p_pt", bufs=2))
    p_stat = ctx.enter_context(tc.tile_pool(name="p_stat", bufs=3))
    p_out = ctx.enter_context(tc.tile_pool(name="p_out", bufs=2))

    def emit_score_softmax(lt):
        """Score matmuls + softmax for l tile lt; returns (PT, 1/sum).
        P column j=t'*128+c holds s=(sb//2)*1024+8c+(sb%2)*4+t' (the
        K fold scrambles s; softmax is order-invariant over s)."""
        score_ps = []
        mx4 = p_stat.tile([128, SB], F32, tag="mx4")
        lb, li = divmod(lt, 4)
        lsl = slice(li * 128, (li + 1) * 128)
        # kc-outer: one stationary (qpT chunk) serves all 4 s-blocks --
        # 8 LDWEIGHTS per l-tile instead of 32
        for sb in range(SB):
            sc_mm = ps_score.tile([128, 512], F32, tag="sc")
            score_ps.append(sc_mm)
        for kc in range(KC):
            for sb in range(SB):
                tsl = slice((sb % 2) * 4, (sb % 2) * 4 + 4)
                nc.tensor.matmul(score_ps[sb], qpT[lb][:, kc, lsl],
                                 kT[sb // 2][:, tsl, kc, :],
                                 start=(kc == 0), stop=(kc == KC - 1))
        for sb in range(SB):
            nc.vector.reduce_max(mx4[:, sb:sb + 1], score_ps[sb], axis=AX.X)

        nm = p_stat.tile([128, 1], F32, tag="nm")
        # nm = -(max) + ln(2^10): P scaled by 1024 (normalizer absorbs it)
        nc.vector.reduce_max(nm, mx4, axis=AX.X, negate=True)
        nc.vector.tensor_scalar_add(nm, nm, PSCALE)
        p_sb = p_p.tile([128, S], F16, tag="p")
        ssum4 = p_stat.tile([128, SB], F32, tag="ssum4")
        for sb in range(SB):
            nc.scalar.activation(p_sb[:, sb * 512:(sb + 1) * 512], score_ps[sb],
                                 AF.Exp, bias=nm, scale=1.0,
                                 accum_out=ssum4[:, sb:sb + 1])
        ssum = p_stat.tile([128, 1], F32, tag="ssum")
        nc.vector.reduce_sum(ssum, ssum4, axis=AX.X)
        rinv = p_stat.tile([128, 1], F32, tag="rinv")
        nc.vector.reciprocal(rinv, ssum)
        # PT[s', sc, l'] = P[l', sc*128+s'] -- one batched xbar transpose
        pt = p_pt.tile([128, ST, 128], F16, tag="pt")
        nc.sync.dma_start(pt, p_sb, transpose=True)
        return pt, rinv

    def emit_pv(lt, pt, rinv):
        """P.T-weighted V accumulation, scale, store. PT chunk sc pairs
        with value rows 8p+sc%8 of half sc//8 (= v16 layout exactly)."""
        out_ps = [ps_out.tile([128, 512], F32, tag="o", name=f"ops{lt}_{i}")
                  for i in range(DB)]
        for sc in range(ST):
            for dc in range(DB):
                nc.tensor.matmul(out_ps[dc], pt[:, sc, :],
                                 v16[sc // 8][:, sc % 8, dc * 512:(dc + 1) * 512],
                                 start=(sc == 0), stop=(sc == ST - 1))
        # two l-tiles share one staging tile -> one store per pair, halving
        # the phase-C links in the serial DMA chain
        if lt % 2 == 0:
            emit_pv.o_sb = p_out.tile([128, 2, D], F32, tag="osb",
                                      name=f"osb{_emit.uid}")
            _emit.uid += 1
        o_sb = emit_pv.o_sb
        for dc in range(DB):
            nc.vector.tensor_scalar_mul(o_sb[:, lt % 2, dc * 512:(dc + 1) * 512],
                                        out_ps[dc], rinv)
        if lt % 2 == 1:
            nc.gpsimd.dma_start(
                out[(lt - 1) * 128:(lt + 1) * 128, :].rearrange(
                    "(t p) d -> p t d", p=128),
                o_sb)

    def phase4():
        # proj lb1-3 + their qT PE-transposes interleave into the attention
        # pipeline a few tiles ahead of the first score tile that reads
        # them; lb2/lb3 Q loads reuse staging slots freed by earlier
        # transposes (WAR dep needs the reader emitted first).
        pending = None
        for lt in range(LT):
            cur = emit_score_softmax(lt)
            if pending is not None:
                emit_pv(lt - 1, *pending)
            pending = cur
            if lt == 0:
                tr_q(1, ps_mm, "mm")
                emit_proj(1)
            if lt in (0, 4):
                lb = lt // 4 + 2
                q16[lb] = cast_load(p_qn, "q16",
                                    query[lb * 512:(lb + 1) * 512, :])
            if lt in (2, 6):
                tr_q(lt // 4 + 2, ps_mm, "mm")
            if lt in (5, 9):
                emit_proj(lt // 4 + 1)
        emit_pv(LT - 1, *pending)

    if loop_T:
        with tc.For_i(0, loop_T, 1):
            phase4()
    else:
        phase4()


_CACHE = {}


def _build(reps=1, loop_T=0, loop_all=0):
    key_ = (reps, loop_T, loop_all)
    if key_ in _CACHE:
        return _CACHE[key_]
    nc = bacc.Bacc("TRN2", target_bir_lowering=False, debug=False,
                   num_devices=N_CORES)
    query = nc.dram_tensor("query", [L, D], F32, kind="ExternalInput").ap()
    key = nc.dram_tensor("key", [S, D], F32, kind="ExternalInput").ap()
    value = nc.dram_tensor("value", [S, D], F32, kind="ExternalInput").ap()
    W = nc.dram_tensor("W", [D, D], F32, kind="ExternalInput").ap()
    b = nc.dram_tensor("b", [D], F32, kind="ExternalInput").ap()
    out = nc.dram_tensor("out", [L, D], F32, kind="ExternalOutput").ap()
    tag = None
    loop_T = loop_T or loop_all
    if reps > 1 or loop_T:
        # distinct I/O signature per variant so the neuron compile cache
        # (keyed on HLO structure, not backend_config) can't collide
        tag = nc.dram_tensor("tag", [8, reps * 100 + max(loop_T, 1)], F32,
                             kind="ExternalOutput").ap()
    with tile.TileContext(nc) as tc:
        if loop_all:
            with tc.For_i(0, loop_all, 1):
                with ExitStack() as ctx:
                    _emit(ctx, tc, query, key, value, W, b, out)
        else:
            for _ in range(reps):
                with ExitStack() as ctx:
                    _emit(ctx, tc, query, key, value, W, b, out, loop_T=loop_T)
        if tag is not None:
            with tc.tile_pool(name="tagp", bufs=1) as tp:
                t = tp.tile([8, reps * 100 + max(loop_T, 1)], F32)
                nc.vector.memset(t, 1.0)
                nc.sync.dma_start(tag, t)
    nc.compile()
    _CACHE[key_] = nc
    return nc


def kernel(key, query, value, W, b):
    key = np.ascontiguousarray(np.asarray(key), dtype=np.float32)
    query = np.ascontiguousarray(np.asarray(query), dtype=np.float32)
    value = np.ascontiguousarray(np.asarray(value), dtype=np.float32)
    W = np.ascontiguousarray(np.asarray(W), dtype=np.float32)
    b = np.ascontiguousarray(np.asarray(b), dtype=np.float32)
    nc = _build()
    in_maps = [
        {"query": query[i], "key": key[i], "value": value[i], "W": W, "b": b}
        for i in range(N_CORES)
    ]
    res = bass_utils.run_bass_kernel_spmd(nc, in_maps, core_ids=list(range(N_CORES)))
    return np.stack([res.results[i]["out"] for i in range(N_CORES)], axis=0)


# revision 54
# speedup vs baseline: 1.1426x; 1.0197x over previous
"""BiLinearAttention TRN2 Bass kernel.

Math (per batch element n, data-parallel over 8 NeuronCores):
    q_proj = query @ W.T + b          # [L, D]
    score  = q_proj @ key.T           # [L, S]
    P      = softmax(score, axis=-1)
    out    = P @ value                # [L, D]

Shapes: query/key/value [2048, 1024] f32 per core, W [1024, 1024], b [1024].

Design notes:
  - Single-pass fp16 matmuls everywhere (1 cycle/row on the PE vs 4 for
    fp32). Rounding all operands to fp16 injects ~0.017 std of logit noise
    (numpy-sim on the real inputs), which softmax turns into 2.5e-3 output
    rel err -- an 8x margin under the 2e-2 gate. The earlier 3-pass fp16
    hi/lo split scheme (2.1e-4) spends 2.2x the PE cycles buying accuracy
    that isn't needed. bf16 (8-bit mantissa, ~8x the logit noise) is NOT
    safe here: score std ~45 with top-2 gaps ~11 makes softmax a
    near-argmax and bf16 visibly corrupts the output.
  - PE floor: proj 131072 + score 262144 + PV 262144 = 655360 cycles
    (273 us at 2.4 GHz).
  - The Tile framework serializes ALL DMAs on one chain (each DMA's
    issue waits the previous DMA's completion semaphore -- HW-trace
    verified), and an X-bar DMA transpose costs 4.8-9.6us. So W/Q/K
    reach contraction-major layout via PE transpose matmuls instead
    (1 cycle/row fp16, ~17us on the PE which is otherwise idle during
    prep), packed 4 chunks per fp16 PSUM tile and drained by DVE/ACT.
    Only the per-tile P transpose stays on the SP X-bar (the PE has no
    slack in phase C; concurrent X-bar streams from two HWDGE queues
    corrupt data, so all X-bars stay on SP).
  - All input loads are gpsimd cast-DMAs (f32 HBM -> f16 SBUF in
    flight). K and V fold 8 rows per partition ("(p t) d", 128
    descriptors -- the 1024-slot SWDGE ring never stalls); the s-order
    scramble this causes is absorbed by softmax order-invariance and
    re-paired in PV via v16[sc//8][:, sc%8]. W/Q keep row-per-partition
    order (their row index becomes the linear k/l output order).
  - Schedule: PE does W/Q0/K transposes then proj lb0 (~10us in); score
    tiles start as soon as kT lands. proj lb1-3 + qT transposes for
    lb1-3 interleave into the attention pipeline. Out stores pair two
    l-tiles per DMA to halve phase-C links in the serial DMA chain.
  - Score runs kc-outer so one stationary LDWEIGHTS serves all 4
    s-blocks (8 instead of 32 per l-tile).
  - Softmax over s in [l, s] layout: free-dim reduce_max on DVE, exp on
    ACT reading score PSUM directly, with accum_out producing the
    denominator. P is emitted as fp16 scaled by 2^10 (folded into the
    exp bias; the normalizer absorbs it) to keep the tail of the
    near-one-hot distribution out of fp16 denormals.
  - P tiles X-bar-transposed, P.T @ value in fp16, then
    out = psum * (1/sum) via per-partition tensor_scalar on DVE.
"""

import numpy as np
from contextlib import ExitStack

import concourse.bass as bass
import concourse.tile as tile
from concourse import mybir, bacc, bass_utils
from concourse.masks import make_identity

F32 = mybir.dt.float32
F16 = mybir.dt.float16
AF = mybir.ActivationFunctionType
AX = mybir.AxisListType

N, L, S, D = 8, 2048, 2048, 1024
N_CORES = 8
LT = L // 128       # 16 l tiles
ST = S // 128       # 16 s tiles
KC = D // 128       # 8 contraction chunks (both q and k dims)
SB = S // 512       # 4 score blocks per l tile
LB = L // 512       # 4 l blocks in projection
DB = D // 512       # 2 d blocks in PV

PSCALE = float(np.log(1024.0))


def _emit(ctx: ExitStack, tc: tile.TileContext,
          query, key, value, W, b, out, loop_T=0):
    nc = tc.nc
    _emit.uid = getattr(_emit, "uid", 0)

    base = ctx.enter_context(tc.tile_pool(name="base", bufs=1))
    b_sb = base.tile([128, KC], F32)
    ident = base.tile([128, 128], F16)

    # persistent transposed fp16 operands. K and V load with rows folded
    # contiguously per partition ("(p t) d": partition p holds rows
    # 8p..8p+7 -- ONE descriptor per partition, 128 per DMA, so the
    # 1024-slot SWDGE ring never stalls on them). This scrambles the
    # s-order: softmax is order-invariant over s and PV re-pairs s via
    # v16 slicing (pt chunk sc <-> v16[sc//8][:, sc%8, :]). W and Q keep
    # the row-per-partition "(t p)" layout (512 desc) because their row
    # indices become the k / l output orders, which must stay linear.
    # WT[h]:  [q', kt_in_half, qc, k']  = W[(4h+t)*128+k', qc*128+q']
    # qpT[lb]: [k', kt, l512]           = q_proj[lb*512+l, kt*128+k']
    # kT[h]:  [k', t, kc, c]            = key[h*1024+8c+t, kc*128+k']
    # v16[h]: [s', u, d]                = value[h*1024+8s'+u, d]
    p_w = ctx.enter_context(tc.tile_pool(name="wt", bufs=1))
    WT = [p_w.tile([128, 4, KC, 128], F16, name=f"WT{h}") for h in range(2)]
    p_qp = ctx.enter_context(tc.tile_pool(name="qp", bufs=1))
    qpT = [p_qp.tile([128, KC, 512], F16, name=f"qpT{i}") for i in range(LB)]
    p_kv = ctx.enter_context(tc.tile_pool(name="kv", bufs=1))
    kT = [p_kv.tile([128, 8, KC, 128], F16, name=f"kT{i}") for i in range(2)]
    v16 = [p_kv.tile([128, 8, D], F16, name=f"v16_{i}") for i in range(2)]

    # fp16 natural-layout staging (written by gpsimd cast-loads, consumed
    # by one batched X-bar each). DMA cost structure (cost-model + HW):
    # consecutive same-kind DMAs pipeline gap-free, but every load<->X-bar
    # transition costs ~2.5us of dead DMA time. So prep is ONE run of
    # loads, then ONE run of X-bars, then V.
    p_qn = ctx.enter_context(tc.tile_pool(name="qn", bufs=2))
    p_qtb = ctx.enter_context(tc.tile_pool(name="qtb", bufs=2))
    ps_mm = ctx.enter_context(tc.tile_pool(name="ps_mm", bufs=2, space="PSUM"))

    def cast_load(pool, tag, src_rows, nt=4, fold=False):
        """nt*128 consecutive rows -> [128,nt,D] f16 gpsimd cast-DMA.
        fold=True: rows contiguous per partition (128 descriptors);
        fold=False: row-per-partition tiles (nt/4 * 512 descriptors)."""
        t = pool.tile([128, nt, D], F16, tag=tag, name=f"{tag}{_emit.uid}")
        _emit.uid += 1
        if fold:
            nc.gpsimd.dma_start(t, src_rows.rearrange("(p t) d -> p t d", t=nt))
        else:
            nc.gpsimd.dma_start(t, src_rows.rearrange("(t p) d -> p t d", p=128))
        return t

    qT_tiles = {}

    def pe_transpose(dst4, src, nt, ps_pool, ps_tag):
        """Transpose [128, nt, D] f16 natural tile into [128, nt, KC, 128]
        contraction-major via PE transpose matmuls (1 cycle/row; the PE is
        the one engine with prep slack). 4 chunk-transposes pack one
        [128,4,128] f16 PSUM tile; one copy drains it to SBUF."""
        for t in range(nt):
            for j in range(2):
                ps = ps_pool.tile([128, 4, 128], F16, tag=ps_tag)
                for i in range(4):
                    qc = 4 * j + i
                    nc.tensor.transpose(ps[:, i, :],
                                        src[:, t, qc * 128:(qc + 1) * 128],
                                        ident)
                nc.any.tensor_copy(dst4[:, t, 4 * j:4 * j + 4, :], ps)

    def emit_proj(lb):
        """q_projT[k, l_blk] = sum_q W.T[q, k] @ queryT[q, l_blk]; +b -> f16"""
        qT = qT_tiles[lb]
        for kt in range(KC):
            mm = ps_mm.tile([128, 512], F32, tag="mm")
            for qc in range(KC):
                nc.tensor.matmul(mm, WT[kt // 4][:, kt % 4, qc, :],
                                 qT[:, :, qc, :],
                                 start=(qc == 0), stop=(qc == KC - 1))
            # bias-add on DVE (not ACT Identity): keeps the ACT table on
            # the Exp set so the 1.3us ACT_TABLE_LOAD doesn't recur
            nc.vector.tensor_scalar_add(qpT[lb][:, kt, :], mm,
                                        b_sb[:, kt:kt + 1])

    with tc.tile_pool(name="kn", bufs=2) as p_kn, \
         tc.tile_pool(name="wn", bufs=2) as p_wn, \
         tc.tile_pool(name="ps_tr", bufs=4, space="PSUM") as ps_tr:
        # gpsimd queue: identity (0.6us, feeds the first W transpose),
        # W (gates the PE restart at each loop iteration boundary), Q0,
        # K, then b (first bias is ~25us in), Q1, V
        make_identity(nc, ident)
        w16 = [cast_load(p_wn, "w16", W[h * 512:(h + 1) * 512, :])
               for h in range(2)]
        q16 = {0: cast_load(p_qn, "q16", query[0:512, :])}
        k16 = [cast_load(p_kn, "k16", key[h * 1024:(h + 1) * 1024, :],
                         nt=8, fold=True)
               for h in range(2)]
        nc.gpsimd.dma_start(b_sb, b.rearrange("(t p) -> p t", p=128))
        q16[1] = cast_load(p_qn, "q16", query[512:1024, :])
        for h in range(2):
            nc.gpsimd.dma_start(
                v16[h],
                value[h * 1024:(h + 1) * 1024, :].rearrange("(p t) d -> p t d", t=8))

        # PE transposes + proj: W, Q0 -> proj lb0 -> K -> (score starts)
        def tr_q(lb, ps_pool, ps_tag):
            t = p_qtb.tile([128, 4, KC, 128], F16, tag="qT",
                           name=f"qT{_emit.uid}")
            _emit.uid += 1
            pe_transpose(t, q16[lb], 4, ps_pool, ps_tag)
            qT_tiles[lb] = t

        for h in range(2):
            pe_transpose(WT[h], w16[h], 4, ps_tr, "tr")
        tr_q(0, ps_tr, "tr")
        for h in range(2):
            pe_transpose(kT[h], k16[h], 8, ps_tr, "tr")
        emit_proj(0)

    # ------- phase C: attention over l tiles -------
    ps_score = ctx.enter_context(tc.tile_pool(name="ps_s", bufs=4, space="PSUM"))
    ps_out = ctx.enter_context(tc.tile_pool(name="ps_o", bufs=2, space="PSUM"))
    p_p = ctx.enter_context(tc.tile_pool(name="p_p", bufs=2))
    p_pt = ctx.enter_context(tc.tile_pool(name="p_pt", bufs=2))
    p_stat = ctx.enter_context(tc.tile_pool(name="p_stat", bufs=3))
    p_out = ctx.enter_context(tc.tile_pool(name="p_out", bufs=2))

    def emit_score_softmax(lt):
        """Score matmuls + softmax for l tile lt; returns (PT, 1/sum).
        P column j=t'*128+c holds s=(sb//2)*1024+8c+(sb%2)*4+t' (the
        K fold scrambles s; softmax is order-invariant over s)."""
        score_ps = []
        mx4 = p_stat.tile([128, SB], F32, tag="mx4")
        lb, li = divmod(lt, 4)
        lsl = slice(li * 128, (li + 1) * 128)
        # kc-outer: one stationary (qpT chunk) serves all 4 s-blocks --
        # 8 LDWEIGHTS per l-tile instead of 32
        for sb in range(SB):
            sc_mm = ps_score.tile([128, 512], F32, tag="sc")
            score_ps.append(sc_mm)
        for kc in range(KC):
            for sb in range(SB):
                tsl = slice((sb % 2) * 4, (sb % 2) * 4 + 4)
                nc.tensor.matmul(score_ps[sb], qpT[lb][:, kc, lsl],
                                 kT[sb // 2][:, tsl, kc, :],
                                 start=(kc == 0), stop=(kc == KC - 1))
        for sb in range(SB):
            nc.vector.reduce_max(mx4[:, sb:sb + 1], score_ps[sb], axis=AX.X)

        nm = p_stat.tile([128, 1], F32, tag="nm")
        # nm = -(max) + ln(2^10): P scaled by 1024 (normalizer absorbs it)
        nc.vector.reduce_max(nm, mx4, axis=AX.X, negate=True)
        nc.vector.tensor_scalar_add(nm, nm, PSCALE)
        p_sb = p_p.tile([128, S], F16, tag="p")
        ssum4 = p_stat.tile([128, SB], F32, tag="ssum4")
        for sb in range(SB):
            nc.scalar.activation(p_sb[:, sb * 512:(sb + 1) * 512], score_ps[sb],
                                 AF.Exp, bias=nm, scale=1.0,
                                 accum_out=ssum4[:, sb:sb + 1])
        ssum = p_stat.tile([128, 1], F32, tag="ssum")
        nc.vector.reduce_sum(ssum, ssum4, axis=AX.X)
        rinv = p_stat.tile([128, 1], F32, tag="rinv")
        nc.vector.reciprocal(rinv, ssum)
        # PT[s', sc, l'] = P[l', sc*128+s'] -- one batched xbar transpose
        pt = p_pt.tile([128, ST, 128], F16, tag="pt")
        nc.sync.dma_start(pt, p_sb, transpose=True)
        return pt, rinv

    def emit_pv(lt, pt, rinv):
        """P.T-weighted V accumulation, scale, store. PT chunk sc pairs
        with value rows 8p+sc%8 of half sc//8 (= v16 layout exactly)."""
        out_ps = [ps_out.tile([128, 512], F32, tag="o", name=f"ops{lt}_{i}")
                  for i in range(DB)]
        for sc in range(ST):
            for dc in range(DB):
                nc.tensor.matmul(out_ps[dc], pt[:, sc, :],
                                 v16[sc // 8][:, sc % 8, dc * 512:(dc + 1) * 512],
                                 start=(sc == 0), stop=(sc == ST - 1))
        # two l-tiles share one staging tile -> one store per pair, halving
        # the phase-C links in the serial DMA chain
        if lt % 2 == 0:
            emit_pv.o_sb = p_out.tile([128, 2, D], F32, tag="osb",
                                      name=f"osb{_emit.uid}")
            _emit.uid += 1
        o_sb = emit_pv.o_sb
        for dc in range(DB):
            nc.vector.tensor_scalar_mul(o_sb[:, lt % 2, dc * 512:(dc + 1) * 512],
                                        out_ps[dc], rinv)
        if lt % 2 == 1:
            nc.gpsimd.dma_start(
                out[(lt - 1) * 128:(lt + 1) * 128, :].rearrange(
                    "(t p) d -> p t d", p=128),
                o_sb)

    def phase4():
        # proj lb1-3 + their qT PE-transposes interleave into the attention
        # pipeline a few tiles ahead of the first score tile that reads
        # them; lb2/lb3 Q loads reuse staging slots freed by earlier
        # transposes (WAR dep needs the reader emitted first).
        pending = None
        for lt in range(LT):
            cur = emit_score_softmax(lt)
            if pending is not None:
                emit_pv(lt - 1, *pending)
            pending = cur
            if lt == 0:
                tr_q(1, ps_mm, "mm")
                emit_proj(1)
            if lt in (0, 4):
                lb = lt // 4 + 2
                q16[lb] = cast_load(p_qn, "q16",
                                    query[lb * 512:(lb + 1) * 512, :])
            if lt in (2, 6):
                tr_q(lt // 4 + 2, ps_mm, "mm")
            if lt in (5, 9):
                emit_proj(lt // 4 + 1)
        emit_pv(LT - 1, *pending)

    if loop_T:
        with tc.For_i(0, loop_T, 1):
            phase4()
    else:
        phase4()


_CACHE = {}


def _build(reps=1, loop_T=0, loop_all=0):
    key_ = (reps, loop_T, loop_all)
    if key_ in _CACHE:
        return _CACHE[key_]
    nc = bacc.Bacc("TRN2", target_bir_lowering=False, debug=False,
                   num_devices=N_CORES)
    query = nc.dram_tensor("query", [L, D], F32, kind="ExternalInput").ap()
    key = nc.dram_tensor("key", [S, D], F32, kind="ExternalInput").ap()
    value = nc.dram_tensor("value", [S, D], F32, kind="ExternalInput").ap()
    W = nc.dram_tensor("W", [D, D], F32, kind="ExternalInput").ap()
    b = nc.dram_tensor("b", [D], F32, kind="ExternalInput").ap()
    out = nc.dram_tensor("out", [L, D], F32, kind="ExternalOutput").ap()
    tag = None
    loop_T = loop_T or loop_all
    if reps > 1 or loop_T:
        # distinct I/O signature per variant so the neuron compile cache
        # (keyed on HLO structure, not backend_config) can't collide
        tag = nc.dram_tensor("tag", [8, reps * 100 + max(loop_T, 1)], F32,
                             kind="ExternalOutput").ap()
    with tile.TileContext(nc) as tc:
        if loop_all:
            with tc.For_i(0, loop_all, 1):
                with ExitStack() as ctx:
                    _emit(ctx, tc, query, key, value, W, b, out)
        else:
            for _ in range(reps):
                with ExitStack() as ctx:
                    _emit(ctx, tc, query, key, value, W, b, out, loop_T=loop_T)
        if tag is not None:
            with tc.tile_pool(name="tagp", bufs=1) as tp:
                t = tp.tile([8, reps * 100 + max(loop_T, 1)], F32)
                nc.vector.memset(t, 1.0)
                nc.sync.dma_start(tag, t)
    nc.compile()
    _CACHE[key_] = nc
    return nc


def kernel(key, query, value, W, b):
    key = np.ascontiguousarray(np.asarray(key), dtype=np.float32)
    query = np.ascontiguousarray(np.asarray(query), dtype=np.float32)
    value = np.ascontiguousarray(np.asarray(value), dtype=np.float32)
    W = np.ascontiguousarray(np.asarray(W), dtype=np.float32)
    b = np.ascontiguousarray(np.asarray(b), dtype=np.float32)
    nc = _build()
    in_maps = [
        {"query": query[i], "key": key[i], "value": value[i], "W": W, "b": b}
        for i in range(N_CORES)
    ]
    res = bass_utils.run_bass_kernel_spmd(nc, in_maps, core_ids=list(range(N_CORES)))
    return np.stack([res.results[i]["out"] for i in range(N_CORES)], axis=0)
